# revision 40
# baseline (speedup 1.0000x reference)
"""Single-head attention on 8 Trainium2 NeuronCores.

Contract: kernel(**inputs) takes the FULL inputs (x [8,2048,768], weights,
biases) and returns the FULL output [8,2048,768] (fp32). Sharding:
data-parallel over batch — core b processes batch b.

Per-core dataflow (matmul operands bf16, fp32 PSUM accumulation — bf16 is
the only operand dtype that lets the PE's HAM clock gate reach 2.4 GHz):
  xt   [896,2048]  host-prepped x[b].T, row 768 = ones (bias row), rest 0-pad
  qT   [64,2048]   = wq_pad.T @ xt   (1/sqrt(64) folded into wq/bq on host)
  kT   [64,2048]   = wk_pad.T @ xt
  v    [2048,65]   = xt.T @ wv_pad   (col 64 == 1.0 via bias-row trick)
  PT   [t,s]       = exp(kT.T @ qT)  scores transposed; no max-subtraction
                     (|score| <= ~2 by construction, exp is safe in fp32)
  OTa  [65,s]      = v.T @ PT        row 64 = softmax denominators
  OTn  = OTa * (1/OTa[64]) broadcast (rank-1 ones matmul), row 64 -> 1.0
  out  [s,768]     = OTn.T @ wp_aug  (wp_aug row 64 = bp: bias folded)
"""

import os
import sys

sys.path.insert(0, "/opt/trn_rl_repo")

import numpy as np
import ml_dtypes

BF16 = ml_dtypes.bfloat16

B, S, E = 8, 2048, 768
D = 64
P = 128
EC = 7            # ceil((E+1)/128): 896 padded contraction rows
NT = S // P       # 16 key blocks
SCW = 512         # s-chunk width
NSC = S // SCW    # 4 s-chunks
N_CORES = 8

_CACHE = {}


def _build_bass():
    import concourse.mybir as mybir
    import concourse.tile as tile
    from concourse import bacc

    f32 = mybir.dt.float32
    f32r = mybir.dt.bfloat16  # PE operand dtype: bf16 streams at full clock
    AF = mybir.ActivationFunctionType
    Alu = mybir.AluOpType

    nc = bacc.Bacc("TRN2", target_bir_lowering=False, debug=False,
                   num_devices=N_CORES)

    bf16 = mybir.dt.bfloat16
    xt_d = nc.dram_tensor("xt", [EC * P, S], bf16, kind="ExternalInput")
    wq_d = nc.dram_tensor("wq", [P, EC * D], bf16, kind="ExternalInput")
    wk_d = nc.dram_tensor("wk", [P, EC * D], bf16, kind="ExternalInput")
    wv_d = nc.dram_tensor("wv", [P, EC * (D + 2)], bf16, kind="ExternalInput")
    wp_d = nc.dram_tensor("wp", [D + 1, E], bf16, kind="ExternalInput")
    out_d = nc.dram_tensor("out", [S, E], f32, kind="ExternalOutput")
    DEBUG = bool(int(os.environ.get("ATTN_DEBUG", "0")))
    if DEBUG:
        dbg_qdup = nc.dram_tensor("dbg_qdup", [P, S], f32, kind="ExternalOutput")
        dbg_kdup = nc.dram_tensor("dbg_kdup", [P, S], f32, kind="ExternalOutput")
        dbg_v = nc.dram_tensor("dbg_v", [P, NT * (D + 2)], f32, kind="ExternalOutput")
        dbg_pt = nc.dram_tensor("dbg_pt", [P, 2 * SCW], f32, kind="ExternalOutput")
        dbg_otn = nc.dram_tensor("dbg_otn", [D + 1, SCW], f32, kind="ExternalOutput")
        dbg_otsb = nc.dram_tensor("dbg_otsb", [D + 1, SCW], f32, kind="ExternalOutput")
        dbg_bcsb = nc.dram_tensor("dbg_bcsb", [P, SCW], f32, kind="ExternalOutput")

    from contextlib import ExitStack

    with tile.TileContext(nc) as tc, ExitStack() as ctx:
        consts = ctx.enter_context(tc.tile_pool(name="consts", bufs=1))
        xpool = ctx.enter_context(tc.tile_pool(name="xt", bufs=EC))
        qkpool = ctx.enter_context(tc.tile_pool(name="qk", bufs=2))
        vpool = ctx.enter_context(tc.tile_pool(name="v", bufs=1))
        ptpool = ctx.enter_context(tc.tile_pool(name="pt", bufs=16))
        bcpool = ctx.enter_context(tc.tile_pool(name="bc", bufs=3))
        otnpool = ctx.enter_context(tc.tile_pool(name="otn", bufs=3))
        recpool = ctx.enter_context(tc.tile_pool(name="rec", bufs=3))
        outpool = ctx.enter_context(tc.tile_pool(name="ob", bufs=4))

        # ---- constants / weights into SBUF ----
        ones_f = consts.tile([1, P], f32)
        nc.vector.memset(ones_f[:], 1.0)
        ones_sb = consts.tile([1, P], f32r)
        nc.vector.tensor_copy(ones_sb[:], ones_f[:])
        actwarm = consts.tile([1, P], f32)
        nc.scalar.activation(actwarm[:], ones_f[:], AF.Exp)
        wq_sb = consts.tile([P, EC, D], f32r)
        nc.sync.dma_start(out=wq_sb[:], in_=wq_d[:].bitcast(f32r).rearrange("p (c d) -> p c d", c=EC))
        wk_sb = consts.tile([P, EC, D], f32r)
        nc.sync.dma_start(out=wk_sb[:], in_=wk_d[:].bitcast(f32r).rearrange("p (c d) -> p c d", c=EC))

        # ---- x^T tiles ----
        xt_t = []
        for c in range(EC):
            t = xpool.tile([P, S], f32r, tag="xt")
            eng = (nc.sync, nc.scalar, nc.gpsimd)[c % 3]
            eng.dma_start(out=t[:], in_=xt_d[c * P:(c + 1) * P, :].bitcast(f32r))
            xt_t.append(t)
        wv_sb = consts.tile([P, EC, D + 2], f32r)
        nc.sync.dma_start(out=wv_sb[:], in_=wv_d[:].bitcast(f32r).rearrange("p (c d) -> p c d", c=EC))
        wp_sb = consts.tile([D + 1, E], f32r)
        nc.sync.dma_start(out=wp_sb[:], in_=wp_d[:].bitcast(f32r))

        # ---- packed q|k projection (col-tiled, c-outer so PE tracks DMA) ----
        # psum rows 0:64 = qT, rows 64:128 = kT; the two matmuls per (c, n)
        # land in disjoint column groups and run concurrently on the PE.
        qdup = qkpool.tile([P, S], f32r, tag="qk")
        kdup = qkpool.tile([P, S], f32r, tag="qk")
        with tc.tile_pool(name="psqk", bufs=8, space="PSUM") as psqk_pool:
            for n in range(NSC):
                sl = slice(n * SCW, (n + 1) * SCW)
                acc = {}
                for nm in ("qA", "qB", "kA", "kB"):
                    acc[nm] = psqk_pool.tile([D, SCW], f32, tag="psqk", name=nm)
                for c in range(EC):
                    for (w_sb, lo, hi) in ((wq_sb, "qA", "qB"), (wk_sb, "kA", "kB")):
                        nc.tensor.matmul(
                            acc[lo][:], lhsT=w_sb[0:D, c, :],
                            rhs=xt_t[c][0:D, sl], tile_position=(0, 0),
                            start=(c == 0), stop=(c == EC - 1))
                        nc.tensor.matmul(
                            acc[hi][:], lhsT=w_sb[D:P, c, :],
                            rhs=xt_t[c][D:P, sl], tile_position=(D, 0),
                            start=(c == 0), stop=(c == EC - 1))
                for dup, lo, hi in ((qdup, "qA", "qB"), (kdup, "kA", "kB")):
                    tmp = bcpool.tile([D, SCW], f32, tag="qktmp", name="qktmp")
                    nc.vector.tensor_copy(tmp[:], acc[hi][:])
                    nc.vector.tensor_tensor(dup[0:D, sl], acc[lo][:], tmp[:],
                                            op=Alu.add)
                    # replicate this slice to the upper half right away
                    nc.gpsimd.dma_start(out=dup[D:P, sl], in_=dup[0:D, sl])
        if DEBUG:
            nc.sync.dma_start(out=dbg_qdup[:], in_=qdup[:].bitcast(f32))
            nc.sync.dma_start(out=dbg_kdup[:], in_=kdup[:].bitcast(f32))

        ps1 = ctx.enter_context(tc.tile_pool(name="ps1", bufs=2, space="PSUM"))
        pspt = ctx.enter_context(tc.tile_pool(name="pspt", bufs=2, space="PSUM"))
        psout = ctx.enter_context(tc.tile_pool(name="psout", bufs=2, space="PSUM"))

        # ---- v projection: row-paired halves of the E contraction ----
        v_sb = vpool.tile([P, NT, D + 2], f32r)
        for tb in range(NT):
            psA = ps1.tile([P, D + 2], f32, tag="ps1", name="vA")
            psB = ps1.tile([P, D + 2], f32, tag="ps1", name="vB")
            tsl = slice(tb * P, (tb + 1) * P)
            for c in range(EC):
                nc.tensor.matmul(
                    psA[:], lhsT=xt_t[c][0:D, tsl], rhs=wv_sb[0:D, c, :],
                    tile_position=(0, 0),
                    start=(c == 0), stop=(c == EC - 1))
                nc.tensor.matmul(
                    psB[:], lhsT=xt_t[c][D:P, tsl], rhs=wv_sb[D:P, c, :],
                    tile_position=(D, 0),
                    start=(c == 0), stop=(c == EC - 1))
            vtmp = bcpool.tile([P, D + 2], f32, tag="vtmp")
            nc.vector.tensor_copy(vtmp[:], psB[:])
            nc.vector.tensor_tensor(v_sb[:, tb, :], psA[:], vtmp[:], op=Alu.add)
        if DEBUG:
            nc.sync.dma_start(out=dbg_v[:], in_=v_sb.rearrange("p a b -> p (a b)").bitcast(f32))

        # ---- attention: 4 s-chunks, 3-stage software pipeline ----
        # PE emission order interleaves scores(sc+1), AV(sc) and proj(sc-1)
        # at pair granularity so the PE never starves (keeps HAM at K=8/8).
        pt_tiles = {}
        ot_ps = {}
        otn_sb = {}
        proj_ob = {}

        def scores_pair(sc, pr):
            # two key blocks concurrently: rows 0:64 and 64:128 of the PE
            if pr == 0:
                pt_tiles[sc] = []
            ps = pspt.tile([P, 2 * SCW], f32, tag="pspt")
            ssl = slice(sc * SCW, (sc + 1) * SCW)
            tbA, tbB = 2 * pr, 2 * pr + 1
            nc.tensor.matmul(
                ps[:, 0:SCW],
                lhsT=kdup[0:D, tbA * P:(tbA + 1) * P], rhs=qdup[0:D, ssl],
                tile_position=(0, 0), start=True, stop=True)
            nc.tensor.matmul(
                ps[:, SCW:2 * SCW],
                lhsT=kdup[D:P, tbB * P:(tbB + 1) * P], rhs=qdup[D:P, ssl],
                tile_position=(D, 0), start=True, stop=True)
            pe = ptpool.tile([P, 2 * SCW], f32r, tag="pt")
            nc.scalar.activation(pe[:], ps[:], AF.Exp)
            if DEBUG and sc == 0 and pr == 0:
                nc.sync.dma_start(out=dbg_pt[:], in_=pe[:].bitcast(f32))
            pt_tiles[sc].append(pe)

        def av_pair(sc, pr):
            # split the K=128 contraction into two 64-row halves that run
            # concurrently in disjoint PE row groups, accumulating into two
            # separate PSUM banks (combined on DVE in norm()).
            if pr == 0:
                ot_ps[sc] = (ps1.tile([D + 1, SCW], f32, tag="ps1", name="otA"),
                             ps1.tile([D + 1, SCW], f32, tag="ps1", name="otB"))
            otA, otB = ot_ps[sc]
            pe = pt_tiles[sc][pr]
            for h in range(2):
                tb = 2 * pr + h
                csl = slice(h * SCW, (h + 1) * SCW)
                nc.tensor.matmul(
                    otA[:], lhsT=v_sb[0:D, tb, 0:D + 1], rhs=pe[0:D, csl],
                    tile_position=(0, 0),
                    start=(tb == 0), stop=(tb == NT - 1))
                nc.tensor.matmul(
                    otB[:], lhsT=v_sb[D:P, tb, 0:D + 1], rhs=pe[D:P, csl],
                    tile_position=(D, 0),
                    start=(tb == 0), stop=(tb == NT - 1))

        norm_state = {}

        def norm_pre(sc):
            # DVE-only: combine AV halves and compute 1/sum; nothing here
            # enters the PE queue, so the PE never stalls on this chain.
            del pt_tiles[sc]
            otA, otB = ot_ps.pop(sc)
            otb_sb = bcpool.tile([D + 1, SCW], f32, tag="otb")
            nc.vector.tensor_copy(otb_sb[:], otB[:])
            ot_sb = bcpool.tile([D + 1, SCW], f32, tag="otsb")
            nc.vector.tensor_tensor(ot_sb[:], otA[:], otb_sb[:], op=Alu.add)
            sum_f = recpool.tile([1, SCW], f32, tag="sumf")
            nc.vector.tensor_copy(sum_f[:], ot_sb[D:D + 1, :])
            rec_f = recpool.tile([1, SCW], f32, tag="recf")
            nc.vector.reciprocal_approx_fast(rec_f[:], sum_f[:])
            rec = recpool.tile([1, SCW], f32r, tag="rec")
            nc.vector.tensor_copy(rec[:], rec_f[:])
            norm_state[sc] = (ot_sb, rec)

        def norm_post(sc):
            ot_sb, rec = norm_state.pop(sc)
            bc = psout.tile([P, SCW], f32, tag="psout", name="bc")
            nc.tensor.matmul(bc[:], lhsT=ones_sb[:], rhs=rec[:],
                             start=True, stop=True)
            bc_sb = bcpool.tile([P, SCW], f32, tag="bc")
            nc.vector.tensor_copy(bc_sb[:], bc[:])
            otn = otnpool.tile([D + 1, SCW], f32r, tag="otn")
            nc.vector.tensor_tensor(otn[:], ot_sb[:], bc_sb[:D + 1, :], op=Alu.mult)
            if DEBUG and sc == 0:
                nc.sync.dma_start(out=dbg_otn[:], in_=otn[:].bitcast(f32))
                nc.sync.dma_start(out=dbg_otsb[:], in_=ot_sb[:])
                nc.sync.dma_start(out=dbg_bcsb[:], in_=bc_sb[:])
            otn_sb[sc] = otn

        def proj_unit(sc, u):
            # u in 0..7 -> (s-block, half of E)
            sbi, n = divmod(u, 2)
            otn = otn_sb[sc]
            if n == 0:
                proj_ob[sc] = outpool.tile([P, E], f32, tag="ob", name="ob")
            ob = proj_ob[sc]
            po = psout.tile([P, E // 2], f32, tag="psout")
            nc.tensor.matmul(
                po[:], lhsT=otn[:, sbi * P:(sbi + 1) * P],
                rhs=wp_sb[:, n * (E // 2):(n + 1) * (E // 2)],
                start=True, stop=True)
            osl = slice(n * (E // 2), (n + 1) * (E // 2))
            nc.vector.tensor_copy(ob[:, osl], po[:])
            if n == 1:
                row0 = sc * SCW + sbi * P
                nc.sync.dma_start(out=out_d[row0:row0 + P, :], in_=ob[:])
                if u == 7:
                    del otn_sb[sc]

        # Pipeline: within chunk sc we emit scores(sc+1) + AV(sc) trios;
        # norm_post(sc-1) enters the PE queue two slots in and proj(sc-1)
        # three slots in, so the PE never waits on the norm DVE chain.
        # scores(0) only needs q/k: emit before the v projection so the
        # PE fills the dup-DMA wait with v work and ACT warms its exp
        # tables early.
        for pr in range(NT // 2):
            scores_pair(0, pr)
        emit_v_proj()
        for sc in range(NSC):
            for pr in range(NT // 2):
                if sc + 1 < NSC:
                    scores_pair(sc + 1, pr)
                av_pair(sc, pr)
                if sc >= 1 and pr == 1:
                    norm_post(sc - 1)
                if sc >= 1 and pr >= 3:
                    proj_unit(sc - 1, pr - 3)
            if sc >= 1:
                for u in range(5, 8):
                    proj_unit(sc - 1, u)
            norm_pre(sc)
        norm_post(NSC - 1)
        for u in range(8):
            proj_unit(NSC - 1, u)

    nc.compile()
    return nc


def _prep_weights(Wk, bk, Wv, bv, Wq, bq, Wp, bp):
    EP = EC * P
    wq = np.zeros((EP, D), np.float32)
    wq[:E] = Wq / 8.0
    wq[E] = bq / 8.0
    wk = np.zeros((EP, D), np.float32)
    wk[:E] = Wk
    wk[E] = bk
    wv = np.zeros((EP, D + 2), np.float32)
    wv[:E, :D] = Wv
    wv[E, :D] = bv
    wv[E, D] = 1.0
    wp = np.zeros((D + 1, E), np.float32)
    wp[:D] = Wp
    wp[D] = bp
    def parr(w):
        c = w.shape[1]
        return np.ascontiguousarray(
            w.reshape(EC, P, c).transpose(1, 0, 2).reshape(P, EC * c)).astype(BF16)
    return parr(wq), parr(wk), parr(wv), wp.astype(BF16)


def _install_ntff_hook():
    """Provide the antenv.axon_hooks shim this image lacks so
    run_bass_kernel_spmd(trace=True) can capture NTFF profiles."""
    import types

    if "antenv.axon_hooks" not in sys.modules:
        mod = types.ModuleType("antenv.axon_hooks")
        state = {}
        mod.set_axon_ntff_profile_hook = lambda h: state.__setitem__("h", h)
        mod.get_axon_ntff_profile_hook = lambda: state.get("h")
        sys.modules["antenv.axon_hooks"] = mod
        import antenv

        antenv.axon_hooks = mod
    mod = sys.modules["antenv.axon_hooks"]
    if mod.get_axon_ntff_profile_hook() is None:
        if "/root/.axon_site" not in sys.path:
            sys.path.insert(0, "/root/.axon_site")
        from trn_agent_boot.trn_boot import _ntff_profile_via_ctypes

        hook = _ntff_profile_via_ctypes("/opt/axon/libaxon_pjrt.so")
        if hook is not None:
            mod.set_axon_ntff_profile_hook(hook)


def kernel(x, Wk, bk, Wv, bv, Wq, bq, Wp, bp):
    from concourse import bass_utils
    from concourse.bass_utils import run_bass_kernel_spmd

    if "nc" not in _CACHE:
        _CACHE["nc"] = _build_bass()
    nc = _CACHE["nc"]

    x = np.ascontiguousarray(np.asarray(x, np.float32))
    wq, wk, wv, wp = _prep_weights(
        *(np.asarray(a, np.float32) for a in (Wk, bk, Wv, bv, Wq, bq, Wp, bp)))

    n_run = int(os.environ.get("ATTN_CORES", str(N_CORES)))
    in_maps = []
    for b in range(N_CORES):
        xt = np.zeros((EC * P, S), np.float32)
        xt[:E] = x[b].T
        xt[E] = 1.0
        in_maps.append({"xt": xt.astype(BF16), "wq": wq, "wk": wk, "wv": wv,
                        "wp": wp})

    trace = bool(int(os.environ.get("ATTN_TRACE", "0")))
    if trace:
        _install_ntff_hook()
        bass_utils.upload_artifacts = lambda tmpdir: tmpdir
    res = run_bass_kernel_spmd(nc, in_maps[:n_run], list(range(n_run)), trace=trace,
                               tmpdir="/tmp/attn_trace" if trace else None)
    if trace and res.exec_time_ns is not None:
        print(f"HW exec time: {res.exec_time_ns} ns")
        _CACHE["exec_time_ns"] = res.exec_time_ns
        _CACHE["trace"] = res.instructions_and_trace
    if bool(int(os.environ.get("ATTN_DEBUG", "0"))):
        _CACHE["dbg"] = res.results
    return np.stack([res.results[b % n_run]["out"] for b in range(N_CORES)])


# revision 41
# speedup vs baseline: 1.0170x; 1.0170x over previous
"""Single-head attention on 8 Trainium2 NeuronCores.

Contract: kernel(**inputs) takes the FULL inputs (x [8,2048,768], weights,
biases) and returns the FULL output [8,2048,768] (fp32). Sharding:
data-parallel over batch — core b processes batch b.

Per-core dataflow (matmul operands bf16, fp32 PSUM accumulation — bf16 is
the only operand dtype that lets the PE's HAM clock gate reach 2.4 GHz):
  xt   [896,2048]  host-prepped x[b].T, row 768 = ones (bias row), rest 0-pad
  qT   [64,2048]   = wq_pad.T @ xt   (1/sqrt(64) folded into wq/bq on host)
  kT   [64,2048]   = wk_pad.T @ xt
  v    [2048,65]   = xt.T @ wv_pad   (col 64 == 1.0 via bias-row trick)
  PT   [t,s]       = exp(kT.T @ qT)  scores transposed; no max-subtraction
                     (|score| <= ~2 by construction, exp is safe in fp32)
  OTa  [65,s]      = v.T @ PT        row 64 = softmax denominators
  OTn  = OTa * (1/OTa[64]) broadcast (rank-1 ones matmul), row 64 -> 1.0
  out  [s,768]     = OTn.T @ wp_aug  (wp_aug row 64 = bp: bias folded)
"""

import os
import sys

sys.path.insert(0, "/opt/trn_rl_repo")

import numpy as np
import ml_dtypes

BF16 = ml_dtypes.bfloat16

B, S, E = 8, 2048, 768
D = 64
P = 128
EC = 7            # ceil((E+1)/128): 896 padded contraction rows
NT = S // P       # 16 key blocks
SCW = 512         # s-chunk width
NSC = S // SCW    # 4 s-chunks
N_CORES = 8

_CACHE = {}


def _build_bass():
    import concourse.mybir as mybir
    import concourse.tile as tile
    from concourse import bacc

    f32 = mybir.dt.float32
    f32r = mybir.dt.bfloat16  # PE operand dtype: bf16 streams at full clock
    AF = mybir.ActivationFunctionType
    Alu = mybir.AluOpType

    nc = bacc.Bacc("TRN2", target_bir_lowering=False, debug=False,
                   num_devices=N_CORES)

    bf16 = mybir.dt.bfloat16
    xt_d = nc.dram_tensor("xt", [EC * P, S], bf16, kind="ExternalInput")
    wq_d = nc.dram_tensor("wq", [P, EC * D], bf16, kind="ExternalInput")
    wk_d = nc.dram_tensor("wk", [P, EC * D], bf16, kind="ExternalInput")
    wv_d = nc.dram_tensor("wv", [P, EC * (D + 2)], bf16, kind="ExternalInput")
    wp_d = nc.dram_tensor("wp", [D + 1, E], bf16, kind="ExternalInput")
    out_d = nc.dram_tensor("out", [S, E], f32, kind="ExternalOutput")
    DEBUG = bool(int(os.environ.get("ATTN_DEBUG", "0")))
    if DEBUG:
        dbg_qdup = nc.dram_tensor("dbg_qdup", [P, S], f32, kind="ExternalOutput")
        dbg_kdup = nc.dram_tensor("dbg_kdup", [P, S], f32, kind="ExternalOutput")
        dbg_v = nc.dram_tensor("dbg_v", [P, NT * (D + 2)], f32, kind="ExternalOutput")
        dbg_pt = nc.dram_tensor("dbg_pt", [P, 2 * SCW], f32, kind="ExternalOutput")
        dbg_otn = nc.dram_tensor("dbg_otn", [D + 1, SCW], f32, kind="ExternalOutput")
        dbg_otsb = nc.dram_tensor("dbg_otsb", [D + 1, SCW], f32, kind="ExternalOutput")
        dbg_bcsb = nc.dram_tensor("dbg_bcsb", [P, SCW], f32, kind="ExternalOutput")

    from contextlib import ExitStack

    with tile.TileContext(nc) as tc, ExitStack() as ctx:
        consts = ctx.enter_context(tc.tile_pool(name="consts", bufs=1))
        xpool = ctx.enter_context(tc.tile_pool(name="xt", bufs=EC))
        qkpool = ctx.enter_context(tc.tile_pool(name="qk", bufs=2))
        vpool = ctx.enter_context(tc.tile_pool(name="v", bufs=1))
        ptpool = ctx.enter_context(tc.tile_pool(name="pt", bufs=16))
        bcpool = ctx.enter_context(tc.tile_pool(name="bc", bufs=3))
        otnpool = ctx.enter_context(tc.tile_pool(name="otn", bufs=3))
        recpool = ctx.enter_context(tc.tile_pool(name="rec", bufs=3))
        outpool = ctx.enter_context(tc.tile_pool(name="ob", bufs=4))

        # ---- constants / weights into SBUF ----
        ones_f = consts.tile([1, P], f32)
        nc.vector.memset(ones_f[:], 1.0)
        ones_sb = consts.tile([1, P], f32r)
        nc.vector.tensor_copy(ones_sb[:], ones_f[:])
        wq_sb = consts.tile([P, EC, D], f32r)
        nc.sync.dma_start(out=wq_sb[:], in_=wq_d[:].bitcast(f32r).rearrange("p (c d) -> p c d", c=EC))
        wk_sb = consts.tile([P, EC, D], f32r)
        nc.sync.dma_start(out=wk_sb[:], in_=wk_d[:].bitcast(f32r).rearrange("p (c d) -> p c d", c=EC))

        # ---- x^T tiles ----
        xt_t = []
        for c in range(EC):
            t = xpool.tile([P, S], f32r, tag="xt")
            nc.sync.dma_start(out=t[:], in_=xt_d[c * P:(c + 1) * P, :].bitcast(f32r))
            xt_t.append(t)
        wv_sb = consts.tile([P, EC, D + 2], f32r)
        nc.sync.dma_start(out=wv_sb[:], in_=wv_d[:].bitcast(f32r).rearrange("p (c d) -> p c d", c=EC))
        wp_sb = consts.tile([D + 1, E], f32r)
        nc.sync.dma_start(out=wp_sb[:], in_=wp_d[:].bitcast(f32r))

        # ---- packed q|k projection (col-tiled, c-outer so PE tracks DMA) ----
        # psum rows 0:64 = qT, rows 64:128 = kT; the two matmuls per (c, n)
        # land in disjoint column groups and run concurrently on the PE.
        qdup = qkpool.tile([P, S], f32r, tag="qk")
        kdup = qkpool.tile([P, S], f32r, tag="qk")
        with tc.tile_pool(name="psqk", bufs=8, space="PSUM") as psqk_pool:
            for n in range(NSC):
                sl = slice(n * SCW, (n + 1) * SCW)
                acc = {}
                for nm in ("qA", "qB", "kA", "kB"):
                    acc[nm] = psqk_pool.tile([D, SCW], f32, tag="psqk", name=nm)
                for c in range(EC):
                    for (w_sb, lo, hi) in ((wq_sb, "qA", "qB"), (wk_sb, "kA", "kB")):
                        nc.tensor.matmul(
                            acc[lo][:], lhsT=w_sb[0:D, c, :],
                            rhs=xt_t[c][0:D, sl], tile_position=(0, 0),
                            start=(c == 0), stop=(c == EC - 1))
                        nc.tensor.matmul(
                            acc[hi][:], lhsT=w_sb[D:P, c, :],
                            rhs=xt_t[c][D:P, sl], tile_position=(D, 0),
                            start=(c == 0), stop=(c == EC - 1))
                for dup, lo, hi in ((qdup, "qA", "qB"), (kdup, "kA", "kB")):
                    tmp = bcpool.tile([D, SCW], f32, tag="qktmp", name="qktmp")
                    nc.vector.tensor_copy(tmp[:], acc[hi][:])
                    nc.vector.tensor_tensor(dup[0:D, sl], acc[lo][:], tmp[:],
                                            op=Alu.add)
                    # replicate this slice to the upper half right away
                    nc.gpsimd.dma_start(out=dup[D:P, sl], in_=dup[0:D, sl])
        if DEBUG:
            nc.sync.dma_start(out=dbg_qdup[:], in_=qdup[:].bitcast(f32))
            nc.sync.dma_start(out=dbg_kdup[:], in_=kdup[:].bitcast(f32))

        ps1 = ctx.enter_context(tc.tile_pool(name="ps1", bufs=2, space="PSUM"))
        pspt = ctx.enter_context(tc.tile_pool(name="pspt", bufs=2, space="PSUM"))
        psout = ctx.enter_context(tc.tile_pool(name="psout", bufs=2, space="PSUM"))

        # ---- v projection: row-paired halves of the E contraction ----
        v_sb = vpool.tile([P, NT, D + 2], f32r)
        for tb in range(NT):
            psA = ps1.tile([P, D + 2], f32, tag="ps1", name="vA")
            psB = ps1.tile([P, D + 2], f32, tag="ps1", name="vB")
            tsl = slice(tb * P, (tb + 1) * P)
            for c in range(EC):
                nc.tensor.matmul(
                    psA[:], lhsT=xt_t[c][0:D, tsl], rhs=wv_sb[0:D, c, :],
                    tile_position=(0, 0),
                    start=(c == 0), stop=(c == EC - 1))
                nc.tensor.matmul(
                    psB[:], lhsT=xt_t[c][D:P, tsl], rhs=wv_sb[D:P, c, :],
                    tile_position=(D, 0),
                    start=(c == 0), stop=(c == EC - 1))
            vtmp = bcpool.tile([P, D + 2], f32, tag="vtmp")
            nc.vector.tensor_copy(vtmp[:], psB[:])
            nc.vector.tensor_tensor(v_sb[:, tb, :], psA[:], vtmp[:], op=Alu.add)
        if DEBUG:
            nc.sync.dma_start(out=dbg_v[:], in_=v_sb.rearrange("p a b -> p (a b)").bitcast(f32))

        # ---- attention: 4 s-chunks, 3-stage software pipeline ----
        # PE emission order interleaves scores(sc+1), AV(sc) and proj(sc-1)
        # at pair granularity so the PE never starves (keeps HAM at K=8/8).
        pt_tiles = {}
        ot_ps = {}
        otn_sb = {}
        proj_ob = {}

        def scores_pair(sc, pr):
            # two key blocks concurrently: rows 0:64 and 64:128 of the PE
            if pr == 0:
                pt_tiles[sc] = []
            ps = pspt.tile([P, 2 * SCW], f32, tag="pspt")
            ssl = slice(sc * SCW, (sc + 1) * SCW)
            tbA, tbB = 2 * pr, 2 * pr + 1
            nc.tensor.matmul(
                ps[:, 0:SCW],
                lhsT=kdup[0:D, tbA * P:(tbA + 1) * P], rhs=qdup[0:D, ssl],
                tile_position=(0, 0), start=True, stop=True)
            nc.tensor.matmul(
                ps[:, SCW:2 * SCW],
                lhsT=kdup[D:P, tbB * P:(tbB + 1) * P], rhs=qdup[D:P, ssl],
                tile_position=(D, 0), start=True, stop=True)
            pe = ptpool.tile([P, 2 * SCW], f32r, tag="pt")
            nc.scalar.activation(pe[:], ps[:], AF.Exp)
            if DEBUG and sc == 0 and pr == 0:
                nc.sync.dma_start(out=dbg_pt[:], in_=pe[:].bitcast(f32))
            pt_tiles[sc].append(pe)

        def av_pair(sc, pr):
            # split the K=128 contraction into two 64-row halves that run
            # concurrently in disjoint PE row groups, accumulating into two
            # separate PSUM banks (combined on DVE in norm()).
            if pr == 0:
                ot_ps[sc] = (ps1.tile([D + 1, SCW], f32, tag="ps1", name="otA"),
                             ps1.tile([D + 1, SCW], f32, tag="ps1", name="otB"))
            otA, otB = ot_ps[sc]
            pe = pt_tiles[sc][pr]
            for h in range(2):
                tb = 2 * pr + h
                csl = slice(h * SCW, (h + 1) * SCW)
                nc.tensor.matmul(
                    otA[:], lhsT=v_sb[0:D, tb, 0:D + 1], rhs=pe[0:D, csl],
                    tile_position=(0, 0),
                    start=(tb == 0), stop=(tb == NT - 1))
                nc.tensor.matmul(
                    otB[:], lhsT=v_sb[D:P, tb, 0:D + 1], rhs=pe[D:P, csl],
                    tile_position=(D, 0),
                    start=(tb == 0), stop=(tb == NT - 1))

        norm_state = {}

        def norm_pre(sc):
            # DVE-only: combine AV halves and compute 1/sum; nothing here
            # enters the PE queue, so the PE never stalls on this chain.
            del pt_tiles[sc]
            otA, otB = ot_ps.pop(sc)
            otb_sb = bcpool.tile([D + 1, SCW], f32, tag="otb")
            nc.vector.tensor_copy(otb_sb[:], otB[:])
            ot_sb = bcpool.tile([D + 1, SCW], f32, tag="otsb")
            nc.vector.tensor_tensor(ot_sb[:], otA[:], otb_sb[:], op=Alu.add)
            sum_f = recpool.tile([1, SCW], f32, tag="sumf")
            nc.vector.tensor_copy(sum_f[:], ot_sb[D:D + 1, :])
            rec_f = recpool.tile([1, SCW], f32, tag="recf")
            nc.vector.reciprocal_approx_fast(rec_f[:], sum_f[:])
            rec = recpool.tile([1, SCW], f32r, tag="rec")
            nc.vector.tensor_copy(rec[:], rec_f[:])
            norm_state[sc] = (ot_sb, rec)

        def norm_post(sc):
            ot_sb, rec = norm_state.pop(sc)
            bc = psout.tile([P, SCW], f32, tag="psout", name="bc")
            nc.tensor.matmul(bc[:], lhsT=ones_sb[:], rhs=rec[:],
                             start=True, stop=True)
            bc_sb = bcpool.tile([P, SCW], f32, tag="bc")
            nc.vector.tensor_copy(bc_sb[:], bc[:])
            otn = otnpool.tile([D + 1, SCW], f32r, tag="otn")
            nc.vector.tensor_tensor(otn[:], ot_sb[:], bc_sb[:D + 1, :], op=Alu.mult)
            if DEBUG and sc == 0:
                nc.sync.dma_start(out=dbg_otn[:], in_=otn[:].bitcast(f32))
                nc.sync.dma_start(out=dbg_otsb[:], in_=ot_sb[:])
                nc.sync.dma_start(out=dbg_bcsb[:], in_=bc_sb[:])
            otn_sb[sc] = otn

        def proj_unit(sc, u):
            # u in 0..7 -> (s-block, half of E)
            sbi, n = divmod(u, 2)
            otn = otn_sb[sc]
            if n == 0:
                proj_ob[sc] = outpool.tile([P, E], f32, tag="ob", name="ob")
            ob = proj_ob[sc]
            po = psout.tile([P, E // 2], f32, tag="psout")
            nc.tensor.matmul(
                po[:], lhsT=otn[:, sbi * P:(sbi + 1) * P],
                rhs=wp_sb[:, n * (E // 2):(n + 1) * (E // 2)],
                start=True, stop=True)
            osl = slice(n * (E // 2), (n + 1) * (E // 2))
            nc.vector.tensor_copy(ob[:, osl], po[:])
            if n == 1:
                row0 = sc * SCW + sbi * P
                nc.sync.dma_start(out=out_d[row0:row0 + P, :], in_=ob[:])
                if u == 7:
                    del otn_sb[sc]

        # Pipeline: within chunk sc we emit scores(sc+1) + AV(sc) trios;
        # norm_post(sc-1) enters the PE queue two slots in and proj(sc-1)
        # three slots in, so the PE never waits on the norm DVE chain.
        # scores(0) only needs q/k: emit before the v projection so the
        # PE fills the dup-DMA wait with v work and ACT warms its exp
        # tables early.
        for pr in range(NT // 2):
            scores_pair(0, pr)
        emit_v_proj()
        for sc in range(NSC):
            for pr in range(NT // 2):
                if sc + 1 < NSC:
                    scores_pair(sc + 1, pr)
                av_pair(sc, pr)
                if sc >= 1 and pr == 1:
                    norm_post(sc - 1)
                if sc >= 1 and pr >= 3:
                    proj_unit(sc - 1, pr - 3)
            if sc >= 1:
                for u in range(5, 8):
                    proj_unit(sc - 1, u)
            norm_pre(sc)
        norm_post(NSC - 1)
        for u in range(8):
            proj_unit(NSC - 1, u)

    nc.compile()
    return nc


def _prep_weights(Wk, bk, Wv, bv, Wq, bq, Wp, bp):
    EP = EC * P
    wq = np.zeros((EP, D), np.float32)
    wq[:E] = Wq / 8.0
    wq[E] = bq / 8.0
    wk = np.zeros((EP, D), np.float32)
    wk[:E] = Wk
    wk[E] = bk
    wv = np.zeros((EP, D + 2), np.float32)
    wv[:E, :D] = Wv
    wv[E, :D] = bv
    wv[E, D] = 1.0
    wp = np.zeros((D + 1, E), np.float32)
    wp[:D] = Wp
    wp[D] = bp
    def parr(w):
        c = w.shape[1]
        return np.ascontiguousarray(
            w.reshape(EC, P, c).transpose(1, 0, 2).reshape(P, EC * c)).astype(BF16)
    return parr(wq), parr(wk), parr(wv), wp.astype(BF16)


def _install_ntff_hook():
    """Provide the antenv.axon_hooks shim this image lacks so
    run_bass_kernel_spmd(trace=True) can capture NTFF profiles."""
    import types

    if "antenv.axon_hooks" not in sys.modules:
        mod = types.ModuleType("antenv.axon_hooks")
        state = {}
        mod.set_axon_ntff_profile_hook = lambda h: state.__setitem__("h", h)
        mod.get_axon_ntff_profile_hook = lambda: state.get("h")
        sys.modules["antenv.axon_hooks"] = mod
        import antenv

        antenv.axon_hooks = mod
    mod = sys.modules["antenv.axon_hooks"]
    if mod.get_axon_ntff_profile_hook() is None:
        if "/root/.axon_site" not in sys.path:
            sys.path.insert(0, "/root/.axon_site")
        from trn_agent_boot.trn_boot import _ntff_profile_via_ctypes

        hook = _ntff_profile_via_ctypes("/opt/axon/libaxon_pjrt.so")
        if hook is not None:
            mod.set_axon_ntff_profile_hook(hook)


def kernel(x, Wk, bk, Wv, bv, Wq, bq, Wp, bp):
    from concourse import bass_utils
    from concourse.bass_utils import run_bass_kernel_spmd

    if "nc" not in _CACHE:
        _CACHE["nc"] = _build_bass()
    nc = _CACHE["nc"]

    x = np.ascontiguousarray(np.asarray(x, np.float32))
    wq, wk, wv, wp = _prep_weights(
        *(np.asarray(a, np.float32) for a in (Wk, bk, Wv, bv, Wq, bq, Wp, bp)))

    n_run = int(os.environ.get("ATTN_CORES", str(N_CORES)))
    in_maps = []
    for b in range(N_CORES):
        xt = np.zeros((EC * P, S), np.float32)
        xt[:E] = x[b].T
        xt[E] = 1.0
        in_maps.append({"xt": xt.astype(BF16), "wq": wq, "wk": wk, "wv": wv,
                        "wp": wp})

    trace = bool(int(os.environ.get("ATTN_TRACE", "0")))
    if trace:
        _install_ntff_hook()
        bass_utils.upload_artifacts = lambda tmpdir: tmpdir
    res = run_bass_kernel_spmd(nc, in_maps[:n_run], list(range(n_run)), trace=trace,
                               tmpdir="/tmp/attn_trace" if trace else None)
    if trace and res.exec_time_ns is not None:
        print(f"HW exec time: {res.exec_time_ns} ns")
        _CACHE["exec_time_ns"] = res.exec_time_ns
        _CACHE["trace"] = res.instructions_and_trace
    if bool(int(os.environ.get("ATTN_DEBUG", "0"))):
        _CACHE["dbg"] = res.results
    return np.stack([res.results[b % n_run]["out"] for b in range(N_CORES)])


# revision 42
# speedup vs baseline: 1.0384x; 1.0211x over previous
"""Single-head attention on 8 Trainium2 NeuronCores.

Contract: kernel(**inputs) takes the FULL inputs (x [8,2048,768], weights,
biases) and returns the FULL output [8,2048,768] (fp32). Sharding:
data-parallel over batch — core b processes batch b.

Per-core dataflow (matmul operands bf16, fp32 PSUM accumulation — bf16 is
the only operand dtype that lets the PE's HAM clock gate reach 2.4 GHz):
  xt   [896,2048]  host-prepped x[b].T, row 768 = ones (bias row), rest 0-pad
  qT   [64,2048]   = wq_pad.T @ xt   (1/sqrt(64) folded into wq/bq on host)
  kT   [64,2048]   = wk_pad.T @ xt
  v    [2048,65]   = xt.T @ wv_pad   (col 64 == 1.0 via bias-row trick)
  PT   [t,s]       = exp(kT.T @ qT)  scores transposed; no max-subtraction
                     (|score| <= ~2 by construction, exp is safe in fp32)
  OTa  [65,s]      = v.T @ PT        row 64 = softmax denominators
  OTn  = OTa * (1/OTa[64]) broadcast (rank-1 ones matmul), row 64 -> 1.0
  out  [s,768]     = OTn.T @ wp_aug  (wp_aug row 64 = bp: bias folded)
"""

import os
import sys

sys.path.insert(0, "/opt/trn_rl_repo")

import numpy as np
import ml_dtypes

BF16 = ml_dtypes.bfloat16

B, S, E = 8, 2048, 768
D = 64
P = 128
EC = 7            # ceil((E+1)/128): 896 padded contraction rows
NT = S // P       # 16 key blocks
SCW = 512         # s-chunk width
NSC = S // SCW    # 4 s-chunks
N_CORES = 8

_CACHE = {}


def _build_bass():
    import concourse.mybir as mybir
    import concourse.tile as tile
    from concourse import bacc

    f32 = mybir.dt.float32
    f32r = mybir.dt.bfloat16  # PE operand dtype: bf16 streams at full clock
    AF = mybir.ActivationFunctionType
    Alu = mybir.AluOpType

    nc = bacc.Bacc("TRN2", target_bir_lowering=False, debug=False,
                   num_devices=N_CORES)

    bf16 = mybir.dt.bfloat16
    xt_d = nc.dram_tensor("xt", [EC * P, S], bf16, kind="ExternalInput")
    wq_d = nc.dram_tensor("wq", [P, EC * D], bf16, kind="ExternalInput")
    wk_d = nc.dram_tensor("wk", [P, EC * D], bf16, kind="ExternalInput")
    wv_d = nc.dram_tensor("wv", [P, EC * (D + 2)], bf16, kind="ExternalInput")
    wp_d = nc.dram_tensor("wp", [D + 1, E], bf16, kind="ExternalInput")
    out_d = nc.dram_tensor("out", [S, E], f32, kind="ExternalOutput")
    DEBUG = bool(int(os.environ.get("ATTN_DEBUG", "0")))
    if DEBUG:
        dbg_qdup = nc.dram_tensor("dbg_qdup", [P, S], f32, kind="ExternalOutput")
        dbg_kdup = nc.dram_tensor("dbg_kdup", [P, S], f32, kind="ExternalOutput")
        dbg_v = nc.dram_tensor("dbg_v", [P, NT * (D + 2)], f32, kind="ExternalOutput")
        dbg_pt = nc.dram_tensor("dbg_pt", [P, 2 * SCW], f32, kind="ExternalOutput")
        dbg_otn = nc.dram_tensor("dbg_otn", [D + 1, SCW], f32, kind="ExternalOutput")
        dbg_otsb = nc.dram_tensor("dbg_otsb", [D + 1, SCW], f32, kind="ExternalOutput")
        dbg_bcsb = nc.dram_tensor("dbg_bcsb", [P, SCW], f32, kind="ExternalOutput")

    from contextlib import ExitStack

    with tile.TileContext(nc) as tc, ExitStack() as ctx:
        consts = ctx.enter_context(tc.tile_pool(name="consts", bufs=1))
        xpool = ctx.enter_context(tc.tile_pool(name="xt", bufs=EC))
        qkpool = ctx.enter_context(tc.tile_pool(name="qk", bufs=2))
        vpool = ctx.enter_context(tc.tile_pool(name="v", bufs=1))
        ptpool = ctx.enter_context(tc.tile_pool(name="pt", bufs=16))
        bcpool = ctx.enter_context(tc.tile_pool(name="bc", bufs=3))
        otnpool = ctx.enter_context(tc.tile_pool(name="otn", bufs=3))
        recpool = ctx.enter_context(tc.tile_pool(name="rec", bufs=3))
        outpool = ctx.enter_context(tc.tile_pool(name="ob", bufs=4))

        # ---- constants / weights into SBUF ----
        ones_f = consts.tile([1, P], f32)
        nc.vector.memset(ones_f[:], 1.0)
        ones_sb = consts.tile([1, P], f32r)
        nc.vector.tensor_copy(ones_sb[:], ones_f[:])
        wq_sb = consts.tile([P, EC, D], f32r)
        nc.sync.dma_start(out=wq_sb[:], in_=wq_d[:].bitcast(f32r).rearrange("p (c d) -> p c d", c=EC))
        wk_sb = consts.tile([P, EC, D], f32r)
        nc.sync.dma_start(out=wk_sb[:], in_=wk_d[:].bitcast(f32r).rearrange("p (c d) -> p c d", c=EC))

        # ---- x^T tiles ----
        xt_t = []
        for c in range(EC):
            t = xpool.tile([P, S], f32r, tag="xt")
            nc.sync.dma_start(out=t[:], in_=xt_d[c * P:(c + 1) * P, :].bitcast(f32r))
            xt_t.append(t)
        wv_sb = consts.tile([P, EC, D + 2], f32r)
        nc.sync.dma_start(out=wv_sb[:], in_=wv_d[:].bitcast(f32r).rearrange("p (c d) -> p c d", c=EC))
        wp_sb = consts.tile([D + 1, E], f32r)
        nc.sync.dma_start(out=wp_sb[:], in_=wp_d[:].bitcast(f32r))

        # ---- packed q|k projection (col-tiled, c-outer so PE tracks DMA) ----
        # psum rows 0:64 = qT, rows 64:128 = kT; the two matmuls per (c, n)
        # land in disjoint column groups and run concurrently on the PE.
        qdup = qkpool.tile([P, S], f32r, tag="qk")
        kdup = qkpool.tile([P, S], f32r, tag="qk")
        with tc.tile_pool(name="psqk", bufs=8, space="PSUM") as psqk_pool:
            for n in range(NSC):
                sl = slice(n * SCW, (n + 1) * SCW)
                acc = {}
                for nm in ("qA", "qB", "kA", "kB"):
                    acc[nm] = psqk_pool.tile([D, SCW], f32, tag="psqk", name=nm)
                for c in range(EC):
                    for (w_sb, lo, hi) in ((wq_sb, "qA", "qB"), (wk_sb, "kA", "kB")):
                        nc.tensor.matmul(
                            acc[lo][:], lhsT=w_sb[0:D, c, :],
                            rhs=xt_t[c][0:D, sl], tile_position=(0, 0),
                            start=(c == 0), stop=(c == EC - 1))
                        nc.tensor.matmul(
                            acc[hi][:], lhsT=w_sb[D:P, c, :],
                            rhs=xt_t[c][D:P, sl], tile_position=(D, 0),
                            start=(c == 0), stop=(c == EC - 1))
                for dup, lo, hi in ((qdup, "qA", "qB"), (kdup, "kA", "kB")):
                    tmp = bcpool.tile([D, SCW], f32, tag="qktmp", name="qktmp")
                    nc.vector.tensor_copy(tmp[:], acc[hi][:])
                    nc.vector.tensor_tensor(dup[0:D, sl], acc[lo][:], tmp[:],
                                            op=Alu.add)
                    # replicate this slice to the upper half right away
                    nc.gpsimd.dma_start(out=dup[D:P, sl], in_=dup[0:D, sl])
        if DEBUG:
            nc.sync.dma_start(out=dbg_qdup[:], in_=qdup[:].bitcast(f32))
            nc.sync.dma_start(out=dbg_kdup[:], in_=kdup[:].bitcast(f32))

        ps1 = ctx.enter_context(tc.tile_pool(name="ps1", bufs=2, space="PSUM"))
        pspt = ctx.enter_context(tc.tile_pool(name="pspt", bufs=2, space="PSUM"))
        psout = ctx.enter_context(tc.tile_pool(name="psout", bufs=2, space="PSUM"))

        # ---- v projection: row-paired halves of the E contraction ----
        v_sb = vpool.tile([P, NT, D + 2], f32r)
        for tb in range(NT):
            psA = ps1.tile([P, D + 2], f32, tag="ps1", name="vA")
            psB = ps1.tile([P, D + 2], f32, tag="ps1", name="vB")
            tsl = slice(tb * P, (tb + 1) * P)
            for c in range(EC):
                nc.tensor.matmul(
                    psA[:], lhsT=xt_t[c][0:D, tsl], rhs=wv_sb[0:D, c, :],
                    tile_position=(0, 0),
                    start=(c == 0), stop=(c == EC - 1))
                nc.tensor.matmul(
                    psB[:], lhsT=xt_t[c][D:P, tsl], rhs=wv_sb[D:P, c, :],
                    tile_position=(D, 0),
                    start=(c == 0), stop=(c == EC - 1))
            vtmp = bcpool.tile([P, D + 2], f32, tag="vtmp")
            nc.vector.tensor_copy(vtmp[:], psB[:])
            nc.vector.tensor_tensor(v_sb[:, tb, :], psA[:], vtmp[:], op=Alu.add)
        if DEBUG:
            nc.sync.dma_start(out=dbg_v[:], in_=v_sb.rearrange("p a b -> p (a b)").bitcast(f32))

        # ---- attention: 4 s-chunks, 3-stage software pipeline ----
        # PE emission order interleaves scores(sc+1), AV(sc) and proj(sc-1)
        # at pair granularity so the PE never starves (keeps HAM at K=8/8).
        pt_tiles = {}
        ot_ps = {}
        otn_sb = {}
        proj_ob = {}

        def scores_pair(sc, pr):
            # two key blocks concurrently: rows 0:64 and 64:128 of the PE
            if pr == 0:
                pt_tiles[sc] = []
            ps = pspt.tile([P, 2 * SCW], f32, tag="pspt")
            ssl = slice(sc * SCW, (sc + 1) * SCW)
            tbA, tbB = 2 * pr, 2 * pr + 1
            nc.tensor.matmul(
                ps[:, 0:SCW],
                lhsT=kdup[0:D, tbA * P:(tbA + 1) * P], rhs=qdup[0:D, ssl],
                tile_position=(0, 0), start=True, stop=True)
            nc.tensor.matmul(
                ps[:, SCW:2 * SCW],
                lhsT=kdup[D:P, tbB * P:(tbB + 1) * P], rhs=qdup[D:P, ssl],
                tile_position=(D, 0), start=True, stop=True)
            pe = ptpool.tile([P, 2 * SCW], f32r, tag="pt")
            nc.scalar.activation(pe[:], ps[:], AF.Exp)
            if DEBUG and sc == 0 and pr == 0:
                nc.sync.dma_start(out=dbg_pt[:], in_=pe[:].bitcast(f32))
            pt_tiles[sc].append(pe)

        def av_pair(sc, pr):
            # split the K=128 contraction into two 64-row halves that run
            # concurrently in disjoint PE row groups, accumulating into two
            # separate PSUM banks (combined on DVE in norm()).
            if pr == 0:
                ot_ps[sc] = (ps1.tile([D + 1, SCW], f32, tag="ps1", name="otA"),
                             ps1.tile([D + 1, SCW], f32, tag="ps1", name="otB"))
            otA, otB = ot_ps[sc]
            pe = pt_tiles[sc][pr]
            for h in range(2):
                tb = 2 * pr + h
                csl = slice(h * SCW, (h + 1) * SCW)
                nc.tensor.matmul(
                    otA[:], lhsT=v_sb[0:D, tb, 0:D + 1], rhs=pe[0:D, csl],
                    tile_position=(0, 0),
                    start=(tb == 0), stop=(tb == NT - 1))
                nc.tensor.matmul(
                    otB[:], lhsT=v_sb[D:P, tb, 0:D + 1], rhs=pe[D:P, csl],
                    tile_position=(D, 0),
                    start=(tb == 0), stop=(tb == NT - 1))

        norm_state = {}

        def norm_pre(sc):
            # DVE-only: combine AV halves and compute 1/sum; nothing here
            # enters the PE queue, so the PE never stalls on this chain.
            del pt_tiles[sc]
            otA, otB = ot_ps.pop(sc)
            otb_sb = bcpool.tile([D + 1, SCW], f32, tag="otb")
            nc.vector.tensor_copy(otb_sb[:], otB[:])
            ot_sb = bcpool.tile([D + 1, SCW], f32, tag="otsb")
            nc.vector.tensor_tensor(ot_sb[:], otA[:], otb_sb[:], op=Alu.add)
            sum_f = recpool.tile([1, SCW], f32, tag="sumf")
            nc.vector.tensor_copy(sum_f[:], ot_sb[D:D + 1, :])
            rec_f = recpool.tile([1, SCW], f32, tag="recf")
            nc.vector.reciprocal_approx_fast(rec_f[:], sum_f[:])
            rec = recpool.tile([1, SCW], f32r, tag="rec")
            nc.vector.tensor_copy(rec[:], rec_f[:])
            norm_state[sc] = (ot_sb, rec)

        def norm_post(sc):
            ot_sb, rec = norm_state.pop(sc)
            bc = psout.tile([P, SCW], f32, tag="psout", name="bc")
            nc.tensor.matmul(bc[:], lhsT=ones_sb[:], rhs=rec[:],
                             start=True, stop=True)
            # ot_sb is SBUF, so the multiply can take the broadcast straight
            # from PSUM (one-PSUM-operand rule) — no staging copy needed
            otn = otnpool.tile([D + 1, SCW], f32r, tag="otn")
            nc.vector.tensor_tensor(otn[:], ot_sb[:], bc[:D + 1, :], op=Alu.mult)
            if DEBUG and sc == 0:
                nc.sync.dma_start(out=dbg_otn[:], in_=otn[:].bitcast(f32))
                nc.sync.dma_start(out=dbg_otsb[:], in_=ot_sb[:])
                nc.sync.dma_start(out=dbg_bcsb[:], in_=bc[:])
            otn_sb[sc] = otn

        def proj_unit(sc, u):
            # u in 0..7 -> (s-block, half of E)
            sbi, n = divmod(u, 2)
            otn = otn_sb[sc]
            if n == 0:
                proj_ob[sc] = outpool.tile([P, E], f32, tag="ob", name="ob")
            ob = proj_ob[sc]
            po = psout.tile([P, E // 2], f32, tag="psout")
            nc.tensor.matmul(
                po[:], lhsT=otn[:, sbi * P:(sbi + 1) * P],
                rhs=wp_sb[:, n * (E // 2):(n + 1) * (E // 2)],
                start=True, stop=True)
            osl = slice(n * (E // 2), (n + 1) * (E // 2))
            nc.vector.tensor_copy(ob[:, osl], po[:])
            if n == 1:
                row0 = sc * SCW + sbi * P
                nc.sync.dma_start(out=out_d[row0:row0 + P, :], in_=ob[:])
                if u == 7:
                    del otn_sb[sc]

        # Pipeline: within chunk sc we emit scores(sc+1) + AV(sc) trios;
        # norm_post(sc-1) enters the PE queue two slots in and proj(sc-1)
        # three slots in, so the PE never waits on the norm DVE chain.
        # scores(0) only needs q/k: emit before the v projection so the
        # PE fills the dup-DMA wait with v work and ACT warms its exp
        # tables early.
        for pr in range(NT // 2):
            scores_pair(0, pr)
        emit_v_proj()
        for sc in range(NSC):
            for pr in range(NT // 2):
                if sc + 1 < NSC:
                    scores_pair(sc + 1, pr)
                av_pair(sc, pr)
                if sc >= 1 and pr == 1:
                    norm_post(sc - 1)
                if sc >= 1 and pr >= 3:
                    proj_unit(sc - 1, pr - 3)
            if sc >= 1:
                for u in range(5, 8):
                    proj_unit(sc - 1, u)
            norm_pre(sc)
        norm_post(NSC - 1)
        for u in range(8):
            proj_unit(NSC - 1, u)

    nc.compile()
    return nc


def _prep_weights(Wk, bk, Wv, bv, Wq, bq, Wp, bp):
    EP = EC * P
    wq = np.zeros((EP, D), np.float32)
    wq[:E] = Wq / 8.0
    wq[E] = bq / 8.0
    wk = np.zeros((EP, D), np.float32)
    wk[:E] = Wk
    wk[E] = bk
    wv = np.zeros((EP, D + 2), np.float32)
    wv[:E, :D] = Wv
    wv[E, :D] = bv
    wv[E, D] = 1.0
    wp = np.zeros((D + 1, E), np.float32)
    wp[:D] = Wp
    wp[D] = bp
    def parr(w):
        c = w.shape[1]
        return np.ascontiguousarray(
            w.reshape(EC, P, c).transpose(1, 0, 2).reshape(P, EC * c)).astype(BF16)
    return parr(wq), parr(wk), parr(wv), wp.astype(BF16)


def _install_ntff_hook():
    """Provide the antenv.axon_hooks shim this image lacks so
    run_bass_kernel_spmd(trace=True) can capture NTFF profiles."""
    import types

    if "antenv.axon_hooks" not in sys.modules:
        mod = types.ModuleType("antenv.axon_hooks")
        state = {}
        mod.set_axon_ntff_profile_hook = lambda h: state.__setitem__("h", h)
        mod.get_axon_ntff_profile_hook = lambda: state.get("h")
        sys.modules["antenv.axon_hooks"] = mod
        import antenv

        antenv.axon_hooks = mod
    mod = sys.modules["antenv.axon_hooks"]
    if mod.get_axon_ntff_profile_hook() is None:
        if "/root/.axon_site" not in sys.path:
            sys.path.insert(0, "/root/.axon_site")
        from trn_agent_boot.trn_boot import _ntff_profile_via_ctypes

        hook = _ntff_profile_via_ctypes("/opt/axon/libaxon_pjrt.so")
        if hook is not None:
            mod.set_axon_ntff_profile_hook(hook)


def kernel(x, Wk, bk, Wv, bv, Wq, bq, Wp, bp):
    from concourse import bass_utils
    from concourse.bass_utils import run_bass_kernel_spmd

    if "nc" not in _CACHE:
        _CACHE["nc"] = _build_bass()
    nc = _CACHE["nc"]

    x = np.ascontiguousarray(np.asarray(x, np.float32))
    wq, wk, wv, wp = _prep_weights(
        *(np.asarray(a, np.float32) for a in (Wk, bk, Wv, bv, Wq, bq, Wp, bp)))

    n_run = int(os.environ.get("ATTN_CORES", str(N_CORES)))
    in_maps = []
    for b in range(N_CORES):
        xt = np.zeros((EC * P, S), np.float32)
        xt[:E] = x[b].T
        xt[E] = 1.0
        in_maps.append({"xt": xt.astype(BF16), "wq": wq, "wk": wk, "wv": wv,
                        "wp": wp})

    trace = bool(int(os.environ.get("ATTN_TRACE", "0")))
    if trace:
        _install_ntff_hook()
        bass_utils.upload_artifacts = lambda tmpdir: tmpdir
    res = run_bass_kernel_spmd(nc, in_maps[:n_run], list(range(n_run)), trace=trace,
                               tmpdir="/tmp/attn_trace" if trace else None)
    if trace and res.exec_time_ns is not None:
        print(f"HW exec time: {res.exec_time_ns} ns")
        _CACHE["exec_time_ns"] = res.exec_time_ns
        _CACHE["trace"] = res.instructions_and_trace
    if bool(int(os.environ.get("ATTN_DEBUG", "0"))):
        _CACHE["dbg"] = res.results
    return np.stack([res.results[b % n_run]["out"] for b in range(N_CORES)])
